# revision 26
# baseline (speedup 1.0000x reference)
"""Expert-parallel MoE GLU FFN for Trainium2 (8 NeuronCores, Bass/Tile).

Strategy: one expert per core. The host routes each (token, slot) pair to
its expert's core, pads each expert batch to a common capacity C, and
pre-transposes x / weights so the device kernel needs no on-chip
transposes. Matmuls run in bf16 with fp32 PSUM accumulation; weights stay
resident in SBUF (12 MiB/core) so HBM traffic is weights-once + streamed
activations.

Per core, per token block (8x512 + one 128-granular tail block):
  fc1:  y^T[f,t] = w1t[d,f].T-reduce @ x^T[d,t]   (256 matmuls, N=block)
  act:  a[ff,t]  = (h * sigmoid(g)) * g           (ScalarE Sigmoid + 2 DVE muls)
  fc2:  o^T[dout,t] = w2t[ff,dout] red @ a[ff,t]  (128 matmuls, N=block)
"""

import sys

for _p in ("/opt/trn_rl_repo", "/opt/pypackages"):
    if _p not in sys.path:
        sys.path.append(_p)

import numpy as np
import ml_dtypes

import concourse.bass as bass
import concourse.mybir as mybir
import concourse.tile as tile
from concourse.bass_utils import run_bass_kernel_spmd

BF16 = ml_dtypes.bfloat16

N_CORES = 8
D_MODEL = 1024
D_FF = 2048
TBLK = 512          # tokens per block (one PSUM bank at fp32)
D_BLKS = D_MODEL // 128      # 8  contraction blocks for fc1
FF_BLKS = D_FF // 128        # 16 contraction blocks for fc2 / a tiles
DOUT_BLKS = D_MODEL // 128   # 8  output blocks


def _fix_multiwaits(nc):
    """The walrus build in this env supports at most ONE sync-wait per
    instruction; split extras into single-wait NOPs placed just before the
    instruction on the same engine (same program order => same semantics)."""
    ctr = 0
    for f in nc.m.functions:
        for bb in f.blocks:
            out, changed = [], False
            for inst in bb.instructions:
                si = inst.sync_info
                waits = list(si.on_wait) if (si and si.on_wait) else []
                if len(waits) > 1:
                    changed = True
                    for w in waits[:-1]:
                        nop = mybir.InstNoOp(
                            name=f"mwfix_{ctr}",
                            engine=inst.engine,
                            sync_info=mybir.SyncInfo(on_wait=[w], on_update=[]),
                            bass_nofuse=True,
                        )
                        ctr += 1
                        nc.register_instruction(nop)
                        out.append(nop)
                    si.on_wait = [waits[-1]]
                out.append(inst)
            if changed:
                bb.instructions = out
    return ctr


def block_sizes(max_count):
    """Program block sizes covering max_count tokens: full 512 blocks plus a
    minimal 128-granular tail block (one PSUM-bank matmul each)."""
    n_full = max_count // TBLK
    rem = max_count - n_full * TBLK
    blocks = [TBLK] * n_full
    if rem:
        blocks.append(-(-rem // 128) * 128)
    return blocks


def _light_drain_and_barrier(self, tick_clock, wait_clock):
    """Tile epilogue minus the final all-engine barrier: the barrier after
    the sem clears only orders per-engine stream ends, which NEFF completion
    already requires, and the preamble of any later execution re-clears and
    barriers before the body runs. Saves ~3us of EVSEM butterfly."""
    import bass_rust

    nc = self.nc
    drain_inst = nc.sync.drain()
    wait_clock.add_sem_waits(
        drain_inst.ins, bass_rust.ScopedClock({None: tick_clock.global_clock})
    )
    nc.all_engine_barrier()
    popped = nc._tile_sem_poison_stack.pop()
    assert popped is self._sem_poison
    # bookkeeping only — skip clear_and_free_semaphores' dma_reset (a ~3.8us
    # gpsimd DRAIN) and range clear: the kernel postamble zeroes every
    # semaphore right after this anyway.
    sems = list(self.sems.allocated().values())
    sem_nums = [s.num if hasattr(s, "num") else s for s in sems]
    nc._state.prepend_free_semaphores(sem_nums)
    for poison_set in nc._tile_sem_poison_stack:
        poison_set.update(sem_nums)


def build_bass(blocks):
    """Build the per-core Bass program for the given token-block sizes.

    DMA instruction issue costs ~0.6us of the issuing engine's sequencer per
    dma_start, so inputs are packed host-side into layouts that need few,
    large 2D DMAs:
      w1c: [FF_BLKS, 128, D_BLKS*256]  per-i tile; cols = k-major packs of
           [h_cols(128) | gate_cols(128)] for (i,k)    -> 16 DMAs
      w2c: [128, FF_BLKS*D_MODEL]  cols = i-major packs of dout  -> 1 DMA
      xc:  [128, 8*C]  block-major; block tb spans cols [8*ts, 8*(ts+tn)),
           k-major inside                              -> 1 DMA per block
    Weight/x DMAs issue on Sync, output DMAs on the otherwise-idle GpSimd.
    """
    C = sum(blocks)
    f32 = mybir.dt.float32
    bf16 = mybir.dt.bfloat16

    tile.TileContext._drain_and_barrier = _light_drain_and_barrier

    nc = bass.Bass()
    xc = nc.declare_dram_parameter("xc", [128, D_BLKS * C], bf16, isOutput=False)
    w1c = nc.declare_dram_parameter(
        "w1c", [FF_BLKS, 128, D_BLKS * 256], bf16, isOutput=False
    )
    w2c = nc.declare_dram_parameter(
        "w2c", [128, FF_BLKS * D_MODEL], bf16, isOutput=False
    )
    yt = nc.declare_dram_parameter("yt", [D_MODEL, C], f32, isOutput=True)

    with tile.TileContext(nc) as tc:
        with (
            tc.tile_pool(name="weights", bufs=1) as wpool,
            tc.tile_pool(name="xin", bufs=2) as xpool,
            tc.tile_pool(name="act", bufs=2) as apool,
            tc.tile_pool(name="out", bufs=3) as opool,
            tc.tile_pool(name="psum", bufs=3, space="PSUM") as psum,
            tc.tile_pool(name="psum_o", bufs=2, space="PSUM") as psum_o,
        ):
            # HAM warm-up: dependency-free dummy matmuls fill the otherwise
            # dead ~7us head (waiting for the first x/weight DMAs) with PE
            # activity, so the clock gate is already at 2.4 GHz when the
            # first real matmul issues (saves the ~3.4us cold window).
            warm = apool.tile([128, 128], bf16, tag="warm")
            nc.gpsimd.memset(warm[:], 0.0)
            pwarm = psum_o.tile([128, 128], f32, tag="po")
            for _ in range(32):
                nc.tensor.matmul(pwarm[:], warm[:], warm[:], start=True, stop=True)

            def load_x(ts, tn):
                t = xpool.tile([128, D_BLKS * tn], bf16, tag="x")
                nc.sync.dma_start(
                    t[:], xc[:, D_BLKS * ts:D_BLKS * (ts + tn)]
                )
                return t

            # block 0 loads split so the very first matmul only waits for
            # its own k=0 chunks (~192 KiB), not the full 2 MiB
            tn0 = blocks[0]
            w1_0a = wpool.tile([128, 256], bf16, tag="w1_0a")
            nc.sync.dma_start(w1_0a[:], w1c[0, :, 0:256])
            x0a = xpool.tile([128, tn0], bf16, tag="x0a")
            nc.sync.dma_start(x0a[:], xc[:, 0:tn0])
            x0b = xpool.tile([128, (D_BLKS - 1) * tn0], bf16, tag="x0b")
            nc.sync.dma_start(x0b[:], xc[:, tn0:D_BLKS * tn0])
            w1_0b = wpool.tile([128, (D_BLKS - 1) * 256], bf16, tag="w1_0b")
            nc.sync.dma_start(w1_0b[:], w1c[0, :, 256:D_BLKS * 256])

            # resident weights; fc1 tiles in i (use) order
            w1_sb = [None]
            for i in range(1, FF_BLKS):
                t = wpool.tile([128, D_BLKS * 256], bf16, tag=f"w1_{i}")
                nc.sync.dma_start(t[:], w1c[i])
                w1_sb.append(t)
            w2_sb = wpool.tile([128, FF_BLKS * D_MODEL], bf16, tag="w2")
            nc.sync.dma_start(w2_sb[:], w2c[:])

            def w1h(i, k):
                if i == 0:
                    t, off = (w1_0a, 0) if k == 0 else (w1_0b, (k - 1) * 256)
                    return t[:, off:off + 128]
                return w1_sb[i][:, k * 256:k * 256 + 128]

            def w1g(i, k):
                if i == 0:
                    t, off = (w1_0a, 128) if k == 0 else (w1_0b, (k - 1) * 256 + 128)
                    return t[:, off:off + 128]
                return w1_sb[i][:, k * 256 + 128:(k + 1) * 256]

            ts = 0
            for tb, tn in enumerate(blocks):
                if tb == 0:
                    def xs(k, tn=tn):
                        return x0a[:] if k == 0 else x0b[:, (k - 1) * tn:k * tn]
                else:
                    x_sb = load_x(ts, tn)

                    def xs(k, x_sb=x_sb, tn=tn):
                        return x_sb[:, k * tn:(k + 1) * tn]

                a_sb = []
                for i in range(FF_BLKS):
                    ph = psum.tile([128, tn], f32, tag="ph")
                    for k in range(D_BLKS):
                        nc.tensor.matmul(
                            ph[:], w1h(i, k), xs(k),
                            start=(k == 0), stop=(k == D_BLKS - 1),
                        )
                    pg = psum.tile([128, tn], f32, tag="pg")
                    for k in range(D_BLKS):
                        nc.tensor.matmul(
                            pg[:], w1g(i, k), xs(k),
                            start=(k == 0), stop=(k == D_BLKS - 1),
                        )
                    # a = h * silu(g) = (h * sigmoid(g)) * g, keeping each
                    # DVE op to a single PSUM operand (one PSUM read port)
                    sg = apool.tile([128, tn], f32, tag="sg")
                    nc.scalar.activation(
                        sg[:], pg[:], mybir.ActivationFunctionType.Sigmoid
                    )
                    hs = apool.tile([128, tn], f32, tag="hs")
                    nc.vector.tensor_mul(hs[:], ph[:], sg[:])
                    a = apool.tile([128, tn], bf16, tag=f"a_{i}")
                    nc.vector.tensor_mul(a[:], pg[:], hs[:])
                    a_sb.append(a)

                for j in range(DOUT_BLKS):
                    po = psum_o.tile([128, tn], f32, tag="po")
                    for i in range(FF_BLKS):
                        nc.tensor.matmul(
                            po[:],
                            w2_sb[:, i * D_MODEL + j * 128:i * D_MODEL + (j + 1) * 128],
                            a_sb[i][:],
                            start=(i == 0), stop=(i == FF_BLKS - 1),
                        )
                    o = opool.tile([128, tn], f32, tag="o")
                    nc.scalar.copy(o[:], po[:])
                    # alternate issuing engine so the ~0.6us per-issue
                    # sequencer cost overlaps
                    eng = nc.gpsimd if j % 2 == 0 else nc.scalar
                    eng.dma_start(yt[j * 128:(j + 1) * 128, ts:ts + tn], o[:])
                ts += tn

    _fix_multiwaits(nc)
    return nc


# test harness hooks: test.py sets _RUN_KWARGS = {"trace": True, ...} to
# profile; LAST_RESULT then carries exec_time_ns / trace paths.
_RUN_KWARGS = {}
LAST_RESULT = None

# blocks-tuple -> (runner, out_name); reuses the compiled NEFF across
# kernel() calls so only the first call pays the neuronxcc compile.
_EXEC_CACHE = {}


def _get_runner(blocks):
    key = tuple(blocks)
    if key in _EXEC_CACHE:
        return _EXEC_CACHE[key]
    import jax
    from jax.experimental.shard_map import shard_map
    from jax.sharding import Mesh, PartitionSpec
    from concourse import bass2jax

    nc = build_bass(list(blocks))
    bass2jax.install_neuronx_cc_hook()

    partition_name = (
        nc.partition_id_tensor.name if nc.partition_id_tensor else None
    )
    in_names, out_names, out_avals, zero_shapes = [], [], [], []
    for alloc in nc.m.functions[0].allocations:
        if not isinstance(alloc, mybir.MemoryLocationSet):
            continue
        name = alloc.memorylocations[0].name
        if alloc.kind == "ExternalInput":
            if name != partition_name:
                in_names.append(name)
        elif alloc.kind == "ExternalOutput":
            out_names.append(name)
            shape = tuple(alloc.tensor_shape)
            dtype = mybir.dt.np(alloc.dtype)
            out_avals.append(jax.core.ShapedArray(shape, dtype))
            zero_shapes.append((shape, dtype))
    n_params = len(in_names)
    n_outs = len(out_names)
    all_names = in_names + out_names
    if partition_name is not None:
        all_names = all_names + [partition_name]
    donate = tuple(range(n_params, n_params + n_outs))

    def _body(*args):
        operands = list(args)
        if partition_name is not None:
            operands.append(bass2jax.partition_id_tensor())
        outs = bass2jax._bass_exec_p.bind(
            *operands,
            out_avals=tuple(out_avals),
            in_names=tuple(all_names),
            out_names=tuple(out_names),
            lowering_input_output_aliases=(),
            sim_require_finite=True,
            sim_require_nnan=True,
            nc=nc,
        )
        return tuple(outs)

    devices = jax.devices()[:N_CORES]
    mesh = Mesh(np.asarray(devices), ("core",))
    sharded = jax.jit(
        shard_map(
            _body,
            mesh=mesh,
            in_specs=(PartitionSpec("core"),) * (n_params + n_outs),
            out_specs=(PartitionSpec("core"),) * n_outs,
            check_rep=False,
        ),
        donate_argnums=donate,
        keep_unused=True,
    )

    def runner(in_maps):
        concat_in = [
            np.concatenate([np.asarray(m[name]) for m in in_maps], axis=0)
            for name in in_names
        ]
        concat_zeros = [
            np.zeros((N_CORES * s[0], *s[1:]), dt) for s, dt in zero_shapes
        ]
        out_arrs = sharded(*concat_in, *concat_zeros)
        return [
            {
                name: np.asarray(out_arrs[i]).reshape(
                    N_CORES, *out_avals[i].shape
                )[c]
                for i, name in enumerate(out_names)
            }
            for c in range(N_CORES)
        ]

    _EXEC_CACHE[key] = runner
    return runner


def _route(indices):
    """Group (token, slot) pairs by expert. Returns (order, starts, counts):
    order = pair indices sorted by expert (stable), starts = prefix offsets."""
    flat = np.asarray(indices).reshape(-1).astype(np.int64)
    order = np.argsort(flat, kind="stable")
    counts = np.bincount(flat, minlength=N_CORES)
    starts = np.zeros(N_CORES + 1, dtype=np.int64)
    np.cumsum(counts, out=starts[1:])
    return order, starts, counts


def kernel(x, fc1_weight, fc2_weight, indices, counts):
    x = np.asarray(x)
    fc1_weight = np.asarray(fc1_weight)
    fc2_weight = np.asarray(fc2_weight)
    n_tok, d_model = x.shape
    assert d_model == D_MODEL

    order, starts, cnt = _route(indices)
    top_k = np.asarray(indices).shape[-1]
    blocks = block_sizes(max(128, int(cnt.max())))
    C = sum(blocks)

    xb = x.astype(BF16)
    tok_of_pair = order // top_k

    in_maps = []
    for e in range(N_CORES):
        rows = tok_of_pair[starts[e]:starts[e + 1]]
        xe = np.zeros((C, D_MODEL), dtype=BF16)
        xe[: len(rows)] = xb[rows]
        # xc[p, 8*ts + k*tn + t] = xe[ts+t, k*128+p], per-block k-major
        xct = xe.T.reshape(D_BLKS, 128, C)          # (k, p, t)
        xc = np.empty((128, D_BLKS * C), dtype=BF16)
        ts = 0
        for tn in blocks:
            blk = xct[:, :, ts:ts + tn]             # (k, p, tn)
            xc[:, D_BLKS * ts:D_BLKS * (ts + tn)] = (
                blk.transpose(1, 0, 2).reshape(128, D_BLKS * tn)
            )
            ts += tn
        # w1c[i, p, k*256 + (0:128)] = h cols, ... + (128:256) = gate cols
        w1t = fc1_weight[e].T.astype(BF16)          # (D_MODEL, 2*D_FF) [d, f]
        h = w1t[:, :D_FF].reshape(D_BLKS, 128, FF_BLKS, 128)
        g = w1t[:, D_FF:].reshape(D_BLKS, 128, FF_BLKS, 128)
        w1i = np.concatenate([h, g], axis=-1)       # (k, p, i, 256)
        w1c = np.ascontiguousarray(
            w1i.transpose(2, 1, 0, 3).reshape(FF_BLKS, 128, D_BLKS * 256)
        )
        # w2c[p, i*D_MODEL + dout] = W2[dout, i*128+p]
        w2t = fc2_weight[e].T.astype(BF16)          # (D_FF, D_MODEL) [ff, dout]
        w2c = np.ascontiguousarray(
            w2t.reshape(FF_BLKS, 128, D_MODEL)
            .transpose(1, 0, 2)
            .reshape(128, FF_BLKS * D_MODEL)
        )
        in_maps.append({"xc": xc, "w1c": w1c, "w2c": w2c})

    if _RUN_KWARGS:
        # profiling path (test harness only)
        nc = build_bass(blocks)
        res = run_bass_kernel_spmd(nc, in_maps, list(range(N_CORES)), **_RUN_KWARGS)
        global LAST_RESULT
        LAST_RESULT = res
        results = res.results
    else:
        results = _get_runner(tuple(blocks))(in_maps)

    out = np.zeros((n_tok * top_k, d_model), dtype=np.float32)
    for e in range(N_CORES):
        n_e = int(cnt[e])
        if n_e == 0:
            continue
        yt = np.asarray(results[e]["yt"])  # (D_MODEL, C) f32
        out[order[starts[e]:starts[e + 1]]] = yt.T[:n_e]
    return out


# revision 28
# speedup vs baseline: 1.0046x; 1.0046x over previous
"""Expert-parallel MoE GLU FFN for Trainium2 (8 NeuronCores, Bass/Tile).

Strategy: one expert per core. The host routes each (token, slot) pair to
its expert's core, pads each expert batch to a common capacity C, and
pre-transposes x / weights so the device kernel needs no on-chip
transposes. Matmuls run in bf16 with fp32 PSUM accumulation; weights stay
resident in SBUF (12 MiB/core) so HBM traffic is weights-once + streamed
activations.

Per core, per token block (8x512 + one 128-granular tail block):
  fc1:  y^T[f,t] = w1t[d,f].T-reduce @ x^T[d,t]   (256 matmuls, N=block)
  act:  a[ff,t]  = (h * sigmoid(g)) * g           (ScalarE Sigmoid + 2 DVE muls)
  fc2:  o^T[dout,t] = w2t[ff,dout] red @ a[ff,t]  (128 matmuls, N=block)
"""

import sys

for _p in ("/opt/trn_rl_repo", "/opt/pypackages"):
    if _p not in sys.path:
        sys.path.append(_p)

import numpy as np
import ml_dtypes

import concourse.bass as bass
import concourse.mybir as mybir
import concourse.tile as tile
from concourse.bass_utils import run_bass_kernel_spmd

BF16 = ml_dtypes.bfloat16

N_CORES = 8
D_MODEL = 1024
D_FF = 2048
TBLK = 512          # tokens per block (one PSUM bank at fp32)
D_BLKS = D_MODEL // 128      # 8  contraction blocks for fc1
FF_BLKS = D_FF // 128        # 16 contraction blocks for fc2 / a tiles
DOUT_BLKS = D_MODEL // 128   # 8  output blocks


def _fix_multiwaits(nc):
    """The walrus build in this env supports at most ONE sync-wait per
    instruction; split extras into single-wait NOPs placed just before the
    instruction on the same engine (same program order => same semantics)."""
    ctr = 0
    for f in nc.m.functions:
        for bb in f.blocks:
            out, changed = [], False
            for inst in bb.instructions:
                si = inst.sync_info
                waits = list(si.on_wait) if (si and si.on_wait) else []
                if len(waits) > 1:
                    changed = True
                    for w in waits[:-1]:
                        nop = mybir.InstNoOp(
                            name=f"mwfix_{ctr}",
                            engine=inst.engine,
                            sync_info=mybir.SyncInfo(on_wait=[w], on_update=[]),
                            bass_nofuse=True,
                        )
                        ctr += 1
                        nc.register_instruction(nop)
                        out.append(nop)
                    si.on_wait = [waits[-1]]
                out.append(inst)
            if changed:
                bb.instructions = out
    return ctr


def block_sizes(max_count):
    """Program block sizes covering max_count tokens: full 512 blocks plus a
    minimal 128-granular tail block (one PSUM-bank matmul each)."""
    n_full = max_count // TBLK
    rem = max_count - n_full * TBLK
    blocks = [TBLK] * n_full
    if rem:
        blocks.append(-(-rem // 128) * 128)
    return blocks


def _light_drain_and_barrier(self, tick_clock, wait_clock):
    """Tile epilogue minus the final all-engine barrier: the barrier after
    the sem clears only orders per-engine stream ends, which NEFF completion
    already requires, and the preamble of any later execution re-clears and
    barriers before the body runs. Saves ~3us of EVSEM butterfly."""
    import bass_rust

    nc = self.nc
    drain_inst = nc.sync.drain()
    wait_clock.add_sem_waits(
        drain_inst.ins, bass_rust.ScopedClock({None: tick_clock.global_clock})
    )
    nc.all_engine_barrier()
    popped = nc._tile_sem_poison_stack.pop()
    assert popped is self._sem_poison
    # bookkeeping only — skip clear_and_free_semaphores' dma_reset (a ~3.8us
    # gpsimd DRAIN) and range clear: the kernel postamble zeroes every
    # semaphore right after this anyway.
    sems = list(self.sems.allocated().values())
    sem_nums = [s.num if hasattr(s, "num") else s for s in sems]
    nc._state.prepend_free_semaphores(sem_nums)
    for poison_set in nc._tile_sem_poison_stack:
        poison_set.update(sem_nums)


def build_bass(blocks):
    """Build the per-core Bass program for the given token-block sizes.

    DMA instruction issue costs ~0.6us of the issuing engine's sequencer per
    dma_start, so inputs are packed host-side into layouts that need few,
    large 2D DMAs:
      w1c: [FF_BLKS, 128, D_BLKS*256]  per-i tile; cols = k-major packs of
           [h_cols(128) | gate_cols(128)] for (i,k)    -> 16 DMAs
      w2c: [128, FF_BLKS*D_MODEL]  cols = i-major packs of dout  -> 1 DMA
      xc:  [128, 8*C]  block-major; block tb spans cols [8*ts, 8*(ts+tn)),
           k-major inside                              -> 1 DMA per block
    Weight/x DMAs issue on Sync, output DMAs on the otherwise-idle GpSimd.
    """
    C = sum(blocks)
    f32 = mybir.dt.float32
    bf16 = mybir.dt.bfloat16

    tile.TileContext._drain_and_barrier = _light_drain_and_barrier

    nc = bass.Bass()
    xc = nc.declare_dram_parameter("xc", [128, D_BLKS * C], bf16, isOutput=False)
    w1c = nc.declare_dram_parameter(
        "w1c", [FF_BLKS, 128, D_BLKS * 256], bf16, isOutput=False
    )
    w2c = nc.declare_dram_parameter(
        "w2c", [128, FF_BLKS * D_MODEL], bf16, isOutput=False
    )
    yt = nc.declare_dram_parameter("yt", [D_MODEL, C], f32, isOutput=True)

    with tile.TileContext(nc) as tc:
        with (
            tc.tile_pool(name="weights", bufs=1) as wpool,
            tc.tile_pool(name="xin", bufs=2) as xpool,
            tc.tile_pool(name="act", bufs=2) as apool,
            tc.tile_pool(name="out", bufs=3) as opool,
            tc.tile_pool(name="psum", bufs=3, space="PSUM") as psum,
            tc.tile_pool(name="psum_o", bufs=2, space="PSUM") as psum_o,
        ):
            # HAM warm-up: dependency-free dummy matmuls fill the otherwise
            # dead ~7us head (waiting for the first x/weight DMAs) with PE
            # activity, so the clock gate is already at 2.4 GHz when the
            # first real matmul issues (saves the ~3.4us cold window).
            warm = apool.tile([128, 128], bf16, tag="warm")
            nc.gpsimd.memset(warm[:], 0.0)
            pwarm = psum_o.tile([128, 128], f32, tag="po")
            for _ in range(48):
                nc.tensor.matmul(pwarm[:], warm[:], warm[:], start=True, stop=True)

            # first block's x goes first so the PE can start ASAP
            def load_x(ts, tn):
                t = xpool.tile([128, D_BLKS * tn], bf16, tag="x")
                nc.sync.dma_start(
                    t[:], xc[:, D_BLKS * ts:D_BLKS * (ts + tn)]
                )
                return t

            x_first = load_x(0, blocks[0])

            # resident weights; fc1 tiles in i (use) order
            w1_sb = []
            for i in range(FF_BLKS):
                t = wpool.tile([128, D_BLKS * 256], bf16, tag=f"w1_{i}")
                nc.sync.dma_start(t[:], w1c[i])
                w1_sb.append(t)
            w2_sb = wpool.tile([128, FF_BLKS * D_MODEL], bf16, tag="w2")
            nc.sync.dma_start(w2_sb[:], w2c[:])

            ts = 0
            for tb, tn in enumerate(blocks):
                x_sb = x_first if tb == 0 else load_x(ts, tn)

                a_sb = []
                for i in range(FF_BLKS):
                    ph = psum.tile([128, tn], f32, tag="ph")
                    for k in range(D_BLKS):
                        nc.tensor.matmul(
                            ph[:], w1_sb[i][:, k * 256:k * 256 + 128],
                            x_sb[:, k * tn:(k + 1) * tn],
                            start=(k == 0), stop=(k == D_BLKS - 1),
                        )
                    pg = psum.tile([128, tn], f32, tag="pg")
                    for k in range(D_BLKS):
                        nc.tensor.matmul(
                            pg[:], w1_sb[i][:, k * 256 + 128:(k + 1) * 256],
                            x_sb[:, k * tn:(k + 1) * tn],
                            start=(k == 0), stop=(k == D_BLKS - 1),
                        )
                    # a = h * silu(g) = (h * sigmoid(g)) * g, keeping each
                    # DVE op to a single PSUM operand (one PSUM read port)
                    sg = apool.tile([128, tn], f32, tag="sg")
                    nc.scalar.activation(
                        sg[:], pg[:], mybir.ActivationFunctionType.Sigmoid
                    )
                    hs = apool.tile([128, tn], f32, tag="hs")
                    nc.vector.tensor_mul(hs[:], ph[:], sg[:])
                    a = apool.tile([128, tn], bf16, tag=f"a_{i}")
                    nc.vector.tensor_mul(a[:], pg[:], hs[:])
                    a_sb.append(a)

                for j in range(DOUT_BLKS):
                    po = psum_o.tile([128, tn], f32, tag="po")
                    for i in range(FF_BLKS):
                        nc.tensor.matmul(
                            po[:],
                            w2_sb[:, i * D_MODEL + j * 128:i * D_MODEL + (j + 1) * 128],
                            a_sb[i][:],
                            start=(i == 0), stop=(i == FF_BLKS - 1),
                        )
                    o = opool.tile([128, tn], f32, tag="o")
                    nc.scalar.copy(o[:], po[:])
                    # alternate issuing engine so the ~0.6us per-issue
                    # sequencer cost overlaps
                    eng = nc.gpsimd if j % 2 == 0 else nc.scalar
                    eng.dma_start(yt[j * 128:(j + 1) * 128, ts:ts + tn], o[:])
                ts += tn

    _fix_multiwaits(nc)
    return nc


# test harness hooks: test.py sets _RUN_KWARGS = {"trace": True, ...} to
# profile; LAST_RESULT then carries exec_time_ns / trace paths.
_RUN_KWARGS = {}
LAST_RESULT = None

# blocks-tuple -> (runner, out_name); reuses the compiled NEFF across
# kernel() calls so only the first call pays the neuronxcc compile.
_EXEC_CACHE = {}


def _get_runner(blocks):
    key = tuple(blocks)
    if key in _EXEC_CACHE:
        return _EXEC_CACHE[key]
    import jax
    from jax.experimental.shard_map import shard_map
    from jax.sharding import Mesh, PartitionSpec
    from concourse import bass2jax

    nc = build_bass(list(blocks))
    bass2jax.install_neuronx_cc_hook()

    partition_name = (
        nc.partition_id_tensor.name if nc.partition_id_tensor else None
    )
    in_names, out_names, out_avals, zero_shapes = [], [], [], []
    for alloc in nc.m.functions[0].allocations:
        if not isinstance(alloc, mybir.MemoryLocationSet):
            continue
        name = alloc.memorylocations[0].name
        if alloc.kind == "ExternalInput":
            if name != partition_name:
                in_names.append(name)
        elif alloc.kind == "ExternalOutput":
            out_names.append(name)
            shape = tuple(alloc.tensor_shape)
            dtype = mybir.dt.np(alloc.dtype)
            out_avals.append(jax.core.ShapedArray(shape, dtype))
            zero_shapes.append((shape, dtype))
    n_params = len(in_names)
    n_outs = len(out_names)
    all_names = in_names + out_names
    if partition_name is not None:
        all_names = all_names + [partition_name]
    donate = tuple(range(n_params, n_params + n_outs))

    def _body(*args):
        operands = list(args)
        if partition_name is not None:
            operands.append(bass2jax.partition_id_tensor())
        outs = bass2jax._bass_exec_p.bind(
            *operands,
            out_avals=tuple(out_avals),
            in_names=tuple(all_names),
            out_names=tuple(out_names),
            lowering_input_output_aliases=(),
            sim_require_finite=True,
            sim_require_nnan=True,
            nc=nc,
        )
        return tuple(outs)

    devices = jax.devices()[:N_CORES]
    mesh = Mesh(np.asarray(devices), ("core",))
    sharded = jax.jit(
        shard_map(
            _body,
            mesh=mesh,
            in_specs=(PartitionSpec("core"),) * (n_params + n_outs),
            out_specs=(PartitionSpec("core"),) * n_outs,
            check_rep=False,
        ),
        donate_argnums=donate,
        keep_unused=True,
    )

    def runner(in_maps):
        concat_in = [
            np.concatenate([np.asarray(m[name]) for m in in_maps], axis=0)
            for name in in_names
        ]
        concat_zeros = [
            np.zeros((N_CORES * s[0], *s[1:]), dt) for s, dt in zero_shapes
        ]
        out_arrs = sharded(*concat_in, *concat_zeros)
        return [
            {
                name: np.asarray(out_arrs[i]).reshape(
                    N_CORES, *out_avals[i].shape
                )[c]
                for i, name in enumerate(out_names)
            }
            for c in range(N_CORES)
        ]

    _EXEC_CACHE[key] = runner
    return runner


def _route(indices):
    """Group (token, slot) pairs by expert. Returns (order, starts, counts):
    order = pair indices sorted by expert (stable), starts = prefix offsets."""
    flat = np.asarray(indices).reshape(-1).astype(np.int64)
    order = np.argsort(flat, kind="stable")
    counts = np.bincount(flat, minlength=N_CORES)
    starts = np.zeros(N_CORES + 1, dtype=np.int64)
    np.cumsum(counts, out=starts[1:])
    return order, starts, counts


def kernel(x, fc1_weight, fc2_weight, indices, counts):
    x = np.asarray(x)
    fc1_weight = np.asarray(fc1_weight)
    fc2_weight = np.asarray(fc2_weight)
    n_tok, d_model = x.shape
    assert d_model == D_MODEL

    order, starts, cnt = _route(indices)
    top_k = np.asarray(indices).shape[-1]
    blocks = block_sizes(max(128, int(cnt.max())))
    C = sum(blocks)

    xb = x.astype(BF16)
    tok_of_pair = order // top_k

    in_maps = []
    for e in range(N_CORES):
        rows = tok_of_pair[starts[e]:starts[e + 1]]
        xe = np.zeros((C, D_MODEL), dtype=BF16)
        xe[: len(rows)] = xb[rows]
        # xc[p, 8*ts + k*tn + t] = xe[ts+t, k*128+p], per-block k-major
        xct = xe.T.reshape(D_BLKS, 128, C)          # (k, p, t)
        xc = np.empty((128, D_BLKS * C), dtype=BF16)
        ts = 0
        for tn in blocks:
            blk = xct[:, :, ts:ts + tn]             # (k, p, tn)
            xc[:, D_BLKS * ts:D_BLKS * (ts + tn)] = (
                blk.transpose(1, 0, 2).reshape(128, D_BLKS * tn)
            )
            ts += tn
        # w1c[i, p, k*256 + (0:128)] = h cols, ... + (128:256) = gate cols
        w1t = fc1_weight[e].T.astype(BF16)          # (D_MODEL, 2*D_FF) [d, f]
        h = w1t[:, :D_FF].reshape(D_BLKS, 128, FF_BLKS, 128)
        g = w1t[:, D_FF:].reshape(D_BLKS, 128, FF_BLKS, 128)
        w1i = np.concatenate([h, g], axis=-1)       # (k, p, i, 256)
        w1c = np.ascontiguousarray(
            w1i.transpose(2, 1, 0, 3).reshape(FF_BLKS, 128, D_BLKS * 256)
        )
        # w2c[p, i*D_MODEL + dout] = W2[dout, i*128+p]
        w2t = fc2_weight[e].T.astype(BF16)          # (D_FF, D_MODEL) [ff, dout]
        w2c = np.ascontiguousarray(
            w2t.reshape(FF_BLKS, 128, D_MODEL)
            .transpose(1, 0, 2)
            .reshape(128, FF_BLKS * D_MODEL)
        )
        in_maps.append({"xc": xc, "w1c": w1c, "w2c": w2c})

    if _RUN_KWARGS:
        # profiling path (test harness only)
        nc = build_bass(blocks)
        res = run_bass_kernel_spmd(nc, in_maps, list(range(N_CORES)), **_RUN_KWARGS)
        global LAST_RESULT
        LAST_RESULT = res
        results = res.results
    else:
        results = _get_runner(tuple(blocks))(in_maps)

    out = np.zeros((n_tok * top_k, d_model), dtype=np.float32)
    for e in range(N_CORES):
        n_e = int(cnt[e])
        if n_e == 0:
            continue
        yt = np.asarray(results[e]["yt"])  # (D_MODEL, C) f32
        out[order[starts[e]:starts[e + 1]]] = yt.T[:n_e]
    return out
